# revision 1
# baseline (speedup 1.0000x reference)
"""Trainium2 Bass kernel for per-node multi-head attention (GNN message passing).

Math (per node n):
  q = (h @ Wq + bq).reshape(4, 64);  k, v likewise
  attn = softmax((q @ k.T) / 8, axis=-1)      # [4, 4], across heads
  out  = (attn @ v).reshape(256)

Strategy: pure data parallel over 8 cores (62500 nodes each), node-on-partition
layout (128 nodes per tile).  Per tile:
  PE  : transpose h (bf16), QKV projections (bf16, biases via K=1 ones-row
        matmuls, softmax scale pre-folded into Wq/bq, Wv pre-reordered to
        (d, g) column order so AV products are step-1 innermost)
  ACT : f32->bf16 casts / PSUM->SBUF copies, exp
  DVE : QK pair products (2x bf16), reduce over d, softmax denom/recip/div,
        AV products (2x bf16), add-tree over g -> f32 output
"""

import sys

sys.path.insert(0, "/opt/trn_rl_repo")

import numpy as np
import ml_dtypes

import concourse.bass as bass
import concourse.bacc as bacc
import concourse.tile as tile
from concourse import mybir
from concourse.bass_utils import run_bass_kernel_spmd
from concourse.masks import make_identity

N_CORES = 8
N_TOTAL = 500000
SHARD = N_TOTAL // N_CORES  # 62500
IN = 256
OUT = 256
NH = 4
HD = 64
P = 128

BF16 = mybir.dt.bfloat16
F32 = mybir.dt.float32
ALU = mybir.AluOpType
AX = mybir.AxisListType
ACTF = mybir.ActivationFunctionType


def build_program(shard_rows: int, compile: bool = True) -> bass.Bass:
    # Bacc (not raw Bass): its compile() runs move_matmul_waits_to_ldweights
    # + generate_event_semaphores, which legalize sync waits to the TRN2
    # per-instruction limits (1 wait; EventSemaphore holds 2).
    nc = bacc.Bacc()

    h_ext = nc.declare_dram_parameter("h", [shard_rows, IN], F32, isOutput=False)
    wq_ext = nc.declare_dram_parameter("wq", [IN, OUT], BF16, isOutput=False)
    wk_ext = nc.declare_dram_parameter("wk", [IN, OUT], BF16, isOutput=False)
    wv_ext = nc.declare_dram_parameter("wv", [IN, OUT], BF16, isOutput=False)
    bias_ext = nc.declare_dram_parameter("bias", [3, OUT], BF16, isOutput=False)
    out_ext = nc.declare_dram_parameter("out", [shard_rows, OUT], F32, isOutput=True)

    n_full, tail = divmod(shard_rows, P)
    tiles = [(i, P) for i in range(n_full)]
    if tail:
        tiles.append((n_full, tail))

    with tile.TileContext(nc) as tc:
        with (
            tc.tile_pool(name="consts", bufs=1) as consts,
            tc.tile_pool(name="io", bufs=8) as io,
            tc.tile_pool(name="work", bufs=3) as work,
            tc.tile_pool(name="ps", bufs=2, space="PSUM") as ps,
        ):
            ident = consts.tile([P, P], F32)
            make_identity(nc, ident)

            # Moving operands for the projections: [Kchunk partition, chunk, proj, col]
            # Const DMAs go on the ACT HWDGE ring so the per-tile stream on
            # the SP ring never accumulates cross-lane waits on them (the
            # DIRECT2D DMA instruction supports at most 2 sync waits).
            w_sb = consts.tile([P, 2, 3, OUT], BF16)
            for c in range(2):
                for j, w in enumerate((wq_ext, wk_ext, wv_ext)):
                    nc.scalar.dma_start(
                        out=w_sb[:, c, j], in_=w[c * P : (c + 1) * P, :]
                    )
            bias_sb = consts.tile([1, 3, OUT], BF16)
            nc.scalar.dma_start(out=bias_sb[0:1], in_=bias_ext[:, :])
            ones_sb = consts.tile([1, P], BF16)
            nc.vector.memset(ones_sb, 1.0)

            for i, p in tiles:
                r0 = i * P
                hf = io.tile([p, IN], F32, tag="hf")
                nc.sync.dma_start(out=hf, in_=h_ext[r0 : r0 + p, :])

                # f32 PE transpose straight from the DMA tile (hf has exactly
                # one reader -> the h-in DMA's WAR dep stays a single PE wait);
                # the PSUM->SBUF copy below does the bf16 cast.
                hT = ps.tile([P, 2, p], F32, tag="hT")
                for c in range(2):
                    nc.tensor.transpose(
                        hT[:, c], hf[:, c * P : (c + 1) * P], ident[:p, :p]
                    )
                hTs = work.tile([P, 2, p], BF16, tag="hTs")
                nc.scalar.copy(out=hTs, in_=hT)

                # q+k share one PSUM bank (N=512, one accumulation group);
                # v is its own bank (N=256).
                qkv_ps = ps.tile([p, 3 * OUT], F32, tag="qkv_ps")
                for c in range(2):
                    nc.tensor.matmul(
                        out=qkv_ps[:, 0:512],
                        lhsT=hTs[:, c, :],
                        rhs=w_sb[:, c, 0:2].rearrange("p a b -> p (a b)"),
                        start=(c == 0),
                        stop=False,
                    )
                    nc.tensor.matmul(
                        out=qkv_ps[:, 512:768],
                        lhsT=hTs[:, c, :],
                        rhs=w_sb[:, c, 2],
                        start=(c == 0),
                        stop=False,
                    )
                nc.tensor.matmul(
                    out=qkv_ps[:, 0:512],
                    lhsT=ones_sb[:, :p],
                    rhs=bias_sb[:, 0:2].rearrange("p a b -> p (a b)"),
                    start=False,
                    stop=True,
                )
                nc.tensor.matmul(
                    out=qkv_ps[:, 512:768],
                    lhsT=ones_sb[:, :p],
                    rhs=bias_sb[:, 2],
                    start=False,
                    stop=True,
                )

                qkv = work.tile([p, 3 * OUT], BF16, tag="qkv")
                nc.scalar.copy(out=qkv, in_=qkv_ps)

                # QK products: P1[n, h, g, d] = q[n, h, d] * k[n, g, d]
                p1 = work.tile([p, NH, NH, HD], BF16, tag="p1")
                qb = (
                    qkv[:, 0:256]
                    .rearrange("p (h one d) -> p h one d", h=NH, one=1)
                    .to_broadcast([p, NH, NH, HD])
                )
                kb = (
                    qkv[:, 256:512]
                    .rearrange("p (one g d) -> p one g d", one=1, g=NH)
                    .to_broadcast([p, NH, NH, HD])
                )
                nc.vector.tensor_tensor(out=p1, in0=qb, in1=kb, op=ALU.mult)

                logits = work.tile([p, NH * NH], F32, tag="logits")
                nc.vector.tensor_reduce(
                    out=logits,
                    in_=p1.rearrange("p h g d -> p (h g) d"),
                    axis=AX.X,
                    op=ALU.add,
                )

                ex = work.tile([p, NH * NH], BF16, tag="ex")
                nc.scalar.activation(out=ex, in_=logits, func=ACTF.Exp)

                den = work.tile([p, NH], F32, tag="den")
                nc.vector.tensor_reduce(
                    out=den,
                    in_=ex.rearrange("p (h g) -> p h g", h=NH),
                    axis=AX.X,
                    op=ALU.add,
                )
                rcp = work.tile([p, NH], F32, tag="rcp")
                nc.vector.reciprocal(out=rcp, in_=den)

                attn = work.tile([p, NH, NH], BF16, tag="attn")
                nc.vector.tensor_tensor(
                    out=attn,
                    in0=ex.rearrange("p (h g) -> p h g", h=NH),
                    in1=rcp.rearrange("p (h one) -> p h one", one=1).to_broadcast(
                        [p, NH, NH]
                    ),
                    op=ALU.mult,
                )

                # AV products: P2[n, h, d, g] = attn[n, h, g] * v[n, d, g]
                # (v was projected with (d, g)-reordered columns)
                p2 = work.tile([p, NH, HD, NH], BF16, tag="p2")
                ab = (
                    attn.rearrange("p h (one g) -> p h one g", one=1)
                    .to_broadcast([p, NH, HD, NH])
                )
                vb = (
                    qkv[:, 512:768]
                    .rearrange("p (one d g) -> p one d g", one=1, d=HD)
                    .to_broadcast([p, NH, HD, NH])
                )
                nc.vector.tensor_tensor(out=p2, in0=ab, in1=vb, op=ALU.mult)

                t1 = work.tile([p, NH, HD, 2], BF16, tag="t1")
                nc.vector.tensor_tensor(
                    out=t1, in0=p2[:, :, :, 0:2], in1=p2[:, :, :, 2:4], op=ALU.add
                )
                osb = io.tile([p, OUT], F32, tag="osb")
                nc.vector.tensor_tensor(
                    out=osb.rearrange("p (h d) -> p h d", h=NH),
                    in0=t1[:, :, :, 0],
                    in1=t1[:, :, :, 1],
                    op=ALU.add,
                )

                nc.sync.dma_start(out=out_ext[r0 : r0 + p, :], in_=osb)

    if compile:
        nc.compile()
    return nc


def prepare_weights(Wq, bq, Wk, bk, Wv, bv):
    """Host-side transforms: fold softmax scale into q, reorder Wv/bv to
    (d, g) column order, cast to bf16."""
    scale = 1.0 / np.sqrt(np.float32(HD))
    bf = ml_dtypes.bfloat16
    wq = (np.asarray(Wq, np.float32) * scale).astype(bf)
    wk = np.asarray(Wk, np.float32).astype(bf)
    cols = np.arange(OUT)
    perm = (cols % HD) * NH + cols // HD  # old col (g*64+d) -> new col (d*4+g)
    wv_r = np.empty((IN, OUT), np.float32)
    wv_r[:, perm] = np.asarray(Wv, np.float32)
    bv_r = np.empty((OUT,), np.float32)
    bv_r[perm] = np.asarray(bv, np.float32)
    bias = np.stack(
        [
            np.asarray(bq, np.float32) * scale,
            np.asarray(bk, np.float32),
            bv_r,
        ]
    ).astype(bf)
    return wq, wk, wv_r.astype(bf), bias


_PROGRAM_CACHE = {}


def _get_program(rows):
    if rows not in _PROGRAM_CACHE:
        _PROGRAM_CACHE[rows] = build_program(rows)
    return _PROGRAM_CACHE[rows]


def kernel(h, Wk, bk, Wq, bq, Wv, bv):
    h = np.ascontiguousarray(np.asarray(h, dtype=np.float32))
    wq, wk, wv, bias = prepare_weights(Wq, bq, Wk, bk, Wv, bv)

    nc = _get_program(SHARD)
    in_maps = []
    for i in range(N_CORES):
        in_maps.append(
            {
                "h": h[i * SHARD : (i + 1) * SHARD],
                "wq": wq,
                "wk": wk,
                "wv": wv,
                "bias": bias,
            }
        )
    res = run_bass_kernel_spmd(nc, in_maps, core_ids=list(range(N_CORES)))
    return np.concatenate([res.results[i]["out"] for i in range(N_CORES)], axis=0)



# revision 3
# speedup vs baseline: 67.5340x; 67.5340x over previous
"""Trainium2 Bass kernel (v6) for per-node multi-head attention (GNN message passing).

Math (per node n):
  q = (h @ Wq + bq).reshape(4, 64);  k, v likewise
  attn = softmax((q @ k.T) / 8, axis=-1)      # [4, 4] across heads
  out  = (attn @ v).reshape(256)

vs baseline:
  - bf16 DRAM output (host upcasts): halves output DMA.
  - h transposed on PE from a bf16 view of the raw f32 tile (high 16 bits
    of each f32 word): kills the f32-rate transpose AND the cast pass.
  - engine rebalance: DVE keeps products/folds/reduce/rcp; Pool runs the
    softmax small-ops and the AV add-tree; ACT does PSUM->SBUF copies + exp.
  - input/output DMAs batched over 4 tiles (amortize HWDGE fixed cost).
  - group-quad instructions: the fold/reduce/exp/softmax/add-tree ops run
    once per 4-tile group on [P, 4, ...] tiles (amortizes per-instruction
    fixed costs ~4x). Products p1/p2 stay per-tile (broadcast AP dims).
  - 3-stage group software pipeline: S0 = projection+products+reduce+exp,
    S1 = den/rcp/attn (lags 1 group), S2 = AV combine + store (lags 2).
"""

import sys

sys.path.insert(0, "/opt/trn_rl_repo")

import numpy as np
import ml_dtypes

import concourse.bass as bass
import concourse.bacc as bacc
import concourse.tile as tile
from concourse import mybir
from concourse.bass_utils import run_bass_kernel_spmd
from concourse.masks import make_identity

N_CORES = 8
N_TOTAL = 500000
SHARD = N_TOTAL // N_CORES  # 62500
IN = 256
OUT = 256
NH = 4
HD = 64
P = 128
GB = 4  # tiles per group (DMA batch + quad instructions)

BF16 = mybir.dt.bfloat16
F32 = mybir.dt.float32
ALU = mybir.AluOpType
AX = mybir.AxisListType
ACTF = mybir.ActivationFunctionType


def build_program(shard_rows: int, compile: bool = True, repeat: int = 1) -> bass.Bass:
    nc = bacc.Bacc()

    h_ext = nc.declare_dram_parameter("h", [shard_rows, IN], F32, isOutput=False)
    wq_ext = nc.declare_dram_parameter("wq", [IN, OUT], BF16, isOutput=False)
    wk_ext = nc.declare_dram_parameter("wk", [IN, OUT], BF16, isOutput=False)
    wv_ext = nc.declare_dram_parameter("wv", [IN, OUT], BF16, isOutput=False)
    bias_ext = nc.declare_dram_parameter("bias", [3, OUT], BF16, isOutput=False)
    out_ext = nc.declare_dram_parameter("out", [shard_rows, OUT], BF16, isOutput=True)

    n_full, tail = divmod(shard_rows, P)
    tiles = [(i, P) for i in range(n_full)]
    if tail:
        tiles.append((n_full, tail))

    # group tiles (tail tile isolated in its own group)
    batches = []
    i = 0
    while i < len(tiles):
        grp = tiles[i : i + GB]
        if grp[-1][1] != P and len(grp) > 1:
            grp = grp[:-1]
        batches.append(grp)
        i += len(grp)

    with tile.TileContext(nc) as tc:
        with (
            tc.tile_pool(name="consts", bufs=1) as consts,
            tc.tile_pool(name="io", bufs=5) as io,
            tc.tile_pool(name="tw", bufs=8) as tw,      # per-tile work
            tc.tile_pool(name="gw", bufs=4) as gw,      # per-group work
            tc.tile_pool(name="sm", bufs=4) as smp,     # softmax (S1 inputs)
            tc.tile_pool(name="av", bufs=5) as avp,     # S2 inputs
            tc.tile_pool(name="ps", bufs=3, space="PSUM") as ps,
            tc.tile_pool(name="pst", bufs=2, space="PSUM") as pst,
        ):
            ident = consts.tile([P, P], BF16)
            make_identity(nc, ident)

            # Weights: [Kchunk partition, chunk, proj, col]; q scaled by 1/8,
            # v reordered to (d, g) column order host-side.
            w_sb = consts.tile([P, 2, 3, OUT], BF16)
            for c in range(2):
                for j, w in enumerate((wq_ext, wk_ext, wv_ext)):
                    nc.scalar.dma_start(
                        out=w_sb[:, c, j], in_=w[c * P : (c + 1) * P, :]
                    )
            bias_sb = consts.tile([1, 3, OUT], BF16)
            nc.scalar.dma_start(out=bias_sb[0:1], in_=bias_ext[:, :])
            ones_sb = consts.tile([1, P], BF16)
            nc.vector.memset(ones_sb, 1.0)

            def dma_in(grp):
                nb = len(grp)
                r0 = grp[0][0] * P
                rows = sum(p for _, p in grp)
                hf = io.tile([P, nb, IN], F32, tag="hf")
                if rows == nb * P:
                    nc.sync.dma_start(
                        out=hf,
                        in_=h_ext[r0 : r0 + rows, :].rearrange(
                            "(b p) f -> p b f", b=nb
                        ),
                    )
                else:
                    nc.sync.dma_start(
                        out=hf[0 : grp[0][1], 0], in_=h_ext[r0 : r0 + rows, :]
                    )
                return hf

            def s0(grp, hf):
                """projection + QK products + fold/reduce/exp (quad ops)."""
                nb = len(grp)
                pmax = grp[0][1] if nb == 1 else P
                p1q = gw.tile([P, nb, NH * NH, HD], BF16, tag="p1q")
                vq = avp.tile([P, nb, OUT], BF16, tag="vq")
                for bi, (ti, p) in enumerate(grp):
                    hfb = (
                        hf[:, bi]
                        .bitcast(BF16)
                        .rearrange("p (f two) -> p f two", two=2)[:, :, 1]
                    )
                    hT = pst.tile([P, 2, P], BF16, tag="hT")
                    for c in range(2):
                        nc.tensor.transpose(
                            hT[:, c, 0:p],
                            hfb[0:p, c * P : (c + 1) * P],
                            ident[0:p, 0:p],
                        )
                    hTs = tw.tile([P, 2, P], BF16, tag="hTs")
                    nc.scalar.copy(out=hTs[:, :, 0:p], in_=hT[:, :, 0:p])

                    qkv_ps = ps.tile([P, 1024], F32, tag="qkv_ps")
                    for c in range(2):
                        nc.tensor.matmul(
                            out=qkv_ps[0:p, 0:512],
                            lhsT=hTs[:, c, 0:p],
                            rhs=w_sb[:, c, 0:2].rearrange("p a b -> p (a b)"),
                            start=(c == 0),
                            stop=False,
                        )
                        nc.tensor.matmul(
                            out=qkv_ps[0:p, 512:768],
                            lhsT=hTs[:, c, 0:p],
                            rhs=w_sb[:, c, 2],
                            start=(c == 0),
                            stop=False,
                        )
                    nc.tensor.matmul(
                        out=qkv_ps[0:p, 0:512],
                        lhsT=ones_sb[:, 0:p],
                        rhs=bias_sb[:, 0:2].rearrange("p a b -> p (a b)"),
                        start=False,
                        stop=True,
                    )
                    nc.tensor.matmul(
                        out=qkv_ps[0:p, 512:768],
                        lhsT=ones_sb[:, 0:p],
                        rhs=bias_sb[:, 2],
                        start=False,
                        stop=True,
                    )

                    qkv = tw.tile([P, 2 * OUT], BF16, tag="qkv")
                    nc.scalar.copy(out=qkv[0:p], in_=qkv_ps[0:p, 0:512])
                    nc.scalar.copy(out=vq[0:p, bi], in_=qkv_ps[0:p, 512:768])

                    # QK products: P1[n, h, g, d] = q[n, h, d] * k[n, g, d]
                    qb = (
                        qkv[0:p, 0:256]
                        .rearrange("p (h one d) -> p h one d", h=NH, one=1)
                        .to_broadcast([p, NH, NH, HD])
                    )
                    kb = (
                        qkv[0:p, 256:512]
                        .rearrange("p (one g d) -> p one g d", one=1, g=NH)
                        .to_broadcast([p, NH, NH, HD])
                    )
                    nc.vector.tensor_tensor(
                        out=p1q[0:p, bi].rearrange("p (h g) d -> p h g d", h=NH),
                        in0=qb,
                        in1=kb,
                        op=ALU.mult,
                    )

                # quad fold d 64->32->16, reduce 16 -> logits, exp
                f1 = gw.tile([P, nb, NH * NH, HD // 2], BF16, tag="f1")
                nc.vector.tensor_tensor(
                    out=f1[0:pmax],
                    in0=p1q[0:pmax, :, :, 0 : HD // 2],
                    in1=p1q[0:pmax, :, :, HD // 2 : HD],
                    op=ALU.add,
                )
                f2 = gw.tile([P, nb, NH * NH, HD // 4], BF16, tag="f2")
                nc.vector.tensor_tensor(
                    out=f2[0:pmax],
                    in0=f1[0:pmax, :, :, 0 : HD // 4],
                    in1=f1[0:pmax, :, :, HD // 4 : HD // 2],
                    op=ALU.add,
                )
                logits = gw.tile([P, nb, NH * NH], F32, tag="logits")
                nc.vector.tensor_reduce(
                    out=logits[0:pmax], in_=f2[0:pmax], axis=AX.X, op=ALU.add
                )
                ex = smp.tile([P, nb, NH * NH], BF16, tag="ex")
                nc.scalar.activation(
                    out=ex[0:pmax], in_=logits[0:pmax], func=ACTF.Exp
                )
                return (pmax, ex, vq)

            def s1(st):
                """softmax normalization (quad): den folds (Pool), rcp (DVE),
                attn (Pool)."""
                pmax, ex, vq = st
                nb = ex.shape[1]
                exh = ex[0:pmax].rearrange("p b (h g) -> p b h g", h=NH)
                den2 = smp.tile([P, nb, NH, 2], F32, tag="den2")
                nc.gpsimd.tensor_tensor(
                    out=den2[0:pmax],
                    in0=exh[:, :, :, 0:2],
                    in1=exh[:, :, :, 2:4],
                    op=ALU.add,
                )
                den = smp.tile([P, nb, NH], F32, tag="den")
                nc.gpsimd.tensor_tensor(
                    out=den[0:pmax],
                    in0=den2[0:pmax, :, :, 0],
                    in1=den2[0:pmax, :, :, 1],
                    op=ALU.add,
                )
                rcp = smp.tile([P, nb, NH], F32, tag="rcp")
                nc.vector.reciprocal(out=rcp[0:pmax], in_=den[0:pmax])

                attn = avp.tile([P, nb, NH, NH], BF16, tag="attn")
                nc.gpsimd.tensor_tensor(
                    out=attn[0:pmax],
                    in0=exh,
                    in1=rcp[0:pmax]
                    .rearrange("p b (h one) -> p b h one", one=1)
                    .to_broadcast([pmax, nb, NH, NH]),
                    op=ALU.mult,
                )
                return (pmax, attn, vq)

            def s2(grp, st, osb):
                """AV combine (p2 per tile, quad add-tree) + store."""
                pmax, attn, vq = st
                nb = len(grp)
                p2q = avp.tile([P, nb, NH, HD, NH], BF16, tag="p2q")
                for bi, (ti, p) in enumerate(grp):
                    ab = (
                        attn[0:p, bi]
                        .rearrange("p h (one g) -> p h one g", one=1)
                        .to_broadcast([p, NH, HD, NH])
                    )
                    vb = (
                        vq[0:p, bi]
                        .rearrange("p (one d g) -> p one d g", one=1, d=HD)
                        .to_broadcast([p, NH, HD, NH])
                    )
                    nc.vector.tensor_tensor(
                        out=p2q[0:p, bi], in0=ab, in1=vb, op=ALU.mult
                    )

                t1 = avp.tile([P, nb, NH, HD, 2], BF16, tag="t1")
                nc.gpsimd.tensor_tensor(
                    out=t1[0:pmax],
                    in0=p2q[0:pmax, :, :, :, 0:2],
                    in1=p2q[0:pmax, :, :, :, 2:4],
                    op=ALU.add,
                )
                nc.gpsimd.tensor_tensor(
                    out=osb[0:pmax].rearrange("p b (h d) -> p b h d", h=NH),
                    in0=t1[0:pmax, :, :, :, 0],
                    in1=t1[0:pmax, :, :, :, 1],
                    op=ALU.add,
                )

                nb = len(grp)
                r0 = grp[0][0] * P
                rows = sum(p for _, p in grp)
                if rows == nb * P:
                    nc.sync.dma_start(
                        out=out_ext[r0 : r0 + rows, :].rearrange(
                            "(b p) f -> p b f", b=nb
                        ),
                        in_=osb,
                    )
                else:
                    nc.sync.dma_start(
                        out=out_ext[r0 : r0 + rows, :], in_=osb[0 : grp[0][1], 0]
                    )

            for _ in range(repeat):
                # group pipeline: s0(g), s1(g-1), s2(g-2)
                ng = len(batches)
                st0s, st1s, osbs = {}, {}, {}
                for gi in range(ng + 2):
                    if gi < ng:
                        grp = batches[gi]
                        hf = dma_in(grp)
                        osb_t = io.tile([P, len(grp), OUT], BF16, tag="osb")
                        osbs[gi] = osb_t
                        st0s[gi] = s0(grp, hf)
                    if 0 <= gi - 1 < ng:
                        st1s[gi - 1] = s1(st0s.pop(gi - 1))
                    if 0 <= gi - 2 < ng:
                        s2(batches[gi - 2], st1s.pop(gi - 2), osbs.pop(gi - 2))

    if compile:
        nc.compile()
    return nc


def prepare_weights(Wq, bq, Wk, bk, Wv, bv):
    """Host-side transforms: fold softmax scale into q, reorder Wv/bv to
    (d, g) column order, cast to bf16."""
    scale = 1.0 / np.sqrt(np.float32(HD))
    bf = ml_dtypes.bfloat16
    wq = (np.asarray(Wq, np.float32) * scale).astype(bf)
    wk = np.asarray(Wk, np.float32).astype(bf)
    cols = np.arange(OUT)
    perm = (cols % HD) * NH + cols // HD  # old col (g*64+d) -> new col (d*4+g)
    wv_r = np.empty((IN, OUT), np.float32)
    wv_r[:, perm] = np.asarray(Wv, np.float32)
    bv_r = np.empty((OUT,), np.float32)
    bv_r[perm] = np.asarray(bv, np.float32)
    bias = np.stack(
        [
            np.asarray(bq, np.float32) * scale,
            np.asarray(bk, np.float32),
            bv_r,
        ]
    ).astype(bf)
    return wq, wk, wv_r.astype(bf), bias


_PROGRAM_CACHE = {}


def _get_program(rows, repeat=1):
    key = (rows, repeat)
    if key not in _PROGRAM_CACHE:
        _PROGRAM_CACHE[key] = build_program(rows, repeat=repeat)
    return _PROGRAM_CACHE[key]


def make_in_maps(inputs):
    h = np.ascontiguousarray(np.asarray(inputs["h"], dtype=np.float32))
    wq, wk, wv, bias = prepare_weights(
        inputs["Wq"], inputs["bq"], inputs["Wk"], inputs["bk"],
        inputs["Wv"], inputs["bv"],
    )
    in_maps = []
    for i in range(N_CORES):
        in_maps.append(
            {
                "h": h[i * SHARD : (i + 1) * SHARD],
                "wq": wq,
                "wk": wk,
                "wv": wv,
                "bias": bias,
            }
        )
    return in_maps


def kernel(h, Wk, bk, Wq, bq, Wv, bv):
    nc = _get_program(SHARD)
    in_maps = make_in_maps(
        {"h": h, "Wk": Wk, "bk": bk, "Wq": Wq, "bq": bq, "Wv": Wv, "bv": bv}
    )
    res = run_bass_kernel_spmd(nc, in_maps, core_ids=list(range(N_CORES)))
    out = np.concatenate([res.results[i]["out"] for i in range(N_CORES)], axis=0)
    return out.astype(np.float32)
